# revision 17
# baseline (speedup 1.0000x reference)
"""Trainium2 Bass kernel for sheaf Dirichlet energy (ConsistencyBasedLaplacianBuilder).

loss = sum_e || maps[rev(e)] @ x[tgt(e)] - maps[e] @ x[src(e)] ||_F^2

Strategy (edge parallelism across 8 NeuronCores):
  The reference edge set is symmetric: edge e < H (=E/2) has its reverse at
  e + H, so the loss equals 2 * sum_{e<H} ||maps[e+H] x[dst] - maps[e] x[src]||^2.
  Each core takes a contiguous slice of the H half-edges and gathers x rows via
  indirect DMA (dma_gather, int16 indices). x is split into two 25000-row
  tables (lo/hi); to keep one table per gather call, each core's edges are
  host-partitioned into 4 static regions by (dst>=25000, src>=25000) bucket,
  each region padded to 13 "pairs" of 16 tiles (2048 edges). Same-table
  regions (0,3) use one interleaved 4096-row gather per pair and a fused
  3-op vector contraction per tile:
    prod[e, i, jj, f] = mcat[e, i, jj] * xcat[e, jj, f]      (f broadcast)
    diff[e, (i f)]    = sum_jj prod[e, i, jj, f]             (strided reduce)
    acc[e, tile]      = sum(diff * diff)                     (fused square+sum)
  Mixed-table regions (1,2) use two 2048-row gathers (dst rows, src rows) and
  split the mult into two half-contractions writing one prod buffer.
  mcat interleaves maps_hi with negated maps_lo so the jj-sum forms the
  difference directly. The loop over all 52 pairs is fully unrolled (no For_i
  barriers/drains) with 4-deep gather buffering, alternating 2 SWDGE queues.
  Per-core partial sums are added on the host.
"""

import sys
import types

import numpy as np

sys.path.insert(0, "/opt/trn_rl_repo")

N = 50000
D = 4
F = 16
DF = D * F            # 64 floats per node row
E = 1600000
H = E // 2            # 800000 undirected pairs
NCORES = 8
EPC = H // NCORES     # 100000 half-edges per core

XS = 25000            # x table split (int16 gather index range)
NPAIR_R = 13          # pairs per bucket region (static layout)
NREG = 4
NPAIRS = NREG * NPAIR_R   # 52
TPP = 16              # tiles per pair
EPP = TPP * 128       # 2048 edges per pair
REG_EDGES = NPAIR_R * EPP  # 26624 slots per region (>= bucket count or fallback)
NT = NPAIRS * TPP     # 832 tiles
NSLOT = NT * 128      # 106496 padded edge slots
XC_DEPTH = 4          # gather buffer pipeline depth


def _inject_axon_hooks():
    """The container's antenv lacks axon_hooks; provide it so NTFF tracing
    (used by test.py, harmless otherwise) can register."""
    if "antenv.axon_hooks" in sys.modules:
        return
    mod = types.ModuleType("antenv.axon_hooks")
    mod._hook = None

    def set_axon_ntff_profile_hook(h):
        mod._hook = h

    def get_axon_ntff_profile_hook():
        return mod._hook

    mod.set_axon_ntff_profile_hook = set_axon_ntff_profile_hook
    mod.get_axon_ntff_profile_hook = get_axon_ntff_profile_hook
    sys.modules["antenv.axon_hooks"] = mod


def _build_program(ncores=NCORES):
    import concourse.bacc as bacc
    import concourse.bass as bass
    import concourse.tile as tile
    from concourse import mybir

    AP = bass.AP
    f32 = mybir.dt.float32
    i16 = mybir.dt.int16
    Op = mybir.AluOpType
    ds = bass.ds

    nc = bacc.Bacc("TRN2", target_bir_lowering=False, debug=False,
                   num_devices=ncores, num_swdge_queues=2)

    xlo_d = nc.dram_tensor("xlo", [XS, DF], f32, kind="ExternalInput")
    xhi_d = nc.dram_tensor("xhi", [XS, DF], f32, kind="ExternalInput")
    mcat_d = nc.dram_tensor("mcat", [128, NT * 32], f32, kind="ExternalInput")
    gidx_d = nc.dram_tensor("gidx", [128, NPAIRS * 256], i16,
                            kind="ExternalInput")
    loss_d = nc.dram_tensor("loss", [1, 1], f32, kind="ExternalOutput")

    with tile.TileContext(nc) as tc, \
         tc.tile_pool(name="persist", bufs=1) as pp, \
         tc.tile_pool(name="gather", bufs=1) as gp, \
         tc.tile_pool(name="work", bufs=4) as wp, \
         tc.tile_pool(name="psum", bufs=1, space="PSUM") as psp:

        mcat_sb = pp.tile([128, NT * 32], f32, tag="mcat")
        gidx_sb = pp.tile([128, NPAIRS * 256], i16, tag="gidx")
        acc = pp.tile([128, NT // 2], f32, tag="acc")

        nc.sync.dma_start(mcat_sb[:], mcat_d[:])
        nc.sync.dma_start(gidx_sb[:], gidx_d[:])

        # negate the maps_lo half in place: columns t*32 + i*8 + (4..7)
        m0 = mcat_sb[:]
        neg_view = AP(m0.tensor, m0.offset + 4,
                      [m0.ap[0], [32, NT], [8, D], [1, 4]])
        nc.vector.tensor_scalar(neg_view, neg_view, -1.0, None, Op.mult)

        xcb = [gp.tile([128, EPP], f32, tag=f"xc{i}", name=f"xc{i}")
               for i in range(XC_DEPTH)]
        tables = (xlo_d, xhi_d)

        def compute_fused(p, xc):
            for k in range(TPP):
                t = p * TPP + k
                if k % 2 == 0:
                    prod = wp.tile([128, 2 * D * 2 * DF], f32, tag="prod",
                                   name="prod")
                    dd2 = wp.tile([128, 2 * DF], f32, tag="dd2", name="dd2")
                xk = xc[:, 128 * k:128 * (k + 1)]
                in0 = AP(xk.tensor, xk.offset,
                         [xk.ap[0], [0, D], [F, 2 * D], [1, F]])
                mk = mcat_sb[:, 32 * t:32 * (t + 1)]
                in1 = AP(mk.tensor, mk.offset,
                         [mk.ap[0], [8, D], [1, 2 * D], [0, F]])
                p0 = prod[:]
                po = p0.offset + (k % 2) * 2 * D * DF
                pout = AP(p0.tensor, po,
                          [p0.ap[0], [2 * DF, D], [F, 2 * D], [1, F]])
                nc.vector.tensor_tensor(pout, in0, in1, Op.mult)
                if k % 2 == 1:
                    pin = AP(p0.tensor, p0.offset,
                             [p0.ap[0], [2 * DF, 2 * D], [1, F], [F, 2 * D]])
                    nc.vector.tensor_reduce(dd2[:], pin,
                                            axis=mybir.AxisListType.X,
                                            op=Op.add)
                    sq2 = wp.tile([128, 2 * DF], f32, tag="sq2", name="sq2")
                    nc.vector.scalar_tensor_tensor(
                        sq2[:], dd2[:], 0.0, dd2[:], Op.bypass, Op.mult,
                        accum_out=acc[:, t // 2:t // 2 + 1])

        def compute_split(p, xc):
            for k in range(TPP):
                t = p * TPP + k
                if k % 2 == 0:
                    prod = wp.tile([128, 2 * D * 2 * DF], f32, tag="prod",
                                   name="prod")
                    dd2 = wp.tile([128, 2 * DF], f32, tag="dd2", name="dd2")
                mk = mcat_sb[:, 32 * t:32 * (t + 1)]
                p0 = prod[:]
                for half, xbase in ((0, 64 * k), (1, TPP * DF + 64 * k)):
                    xk = xc[:, xbase:xbase + DF]
                    in0 = AP(xk.tensor, xk.offset,
                             [xk.ap[0], [0, D], [F, D], [1, F]])
                    in1 = AP(mk.tensor, mk.offset + 4 * half,
                             [mk.ap[0], [8, D], [1, D], [0, F]])
                    pout = AP(p0.tensor,
                              p0.offset + (k % 2) * 2 * D * DF + 4 * F * half,
                              [p0.ap[0], [2 * DF, D], [F, D], [1, F]])
                    nc.vector.tensor_tensor(pout, in0, in1, Op.mult)
                if k % 2 == 1:
                    pin = AP(p0.tensor, p0.offset,
                             [p0.ap[0], [2 * DF, 2 * D], [1, F], [F, 2 * D]])
                    nc.vector.tensor_reduce(dd2[:], pin,
                                            axis=mybir.AxisListType.X,
                                            op=Op.add)
                    sq2 = wp.tile([128, 2 * DF], f32, tag="sq2", name="sq2")
                    nc.vector.scalar_tensor_tensor(
                        sq2[:], dd2[:], 0.0, dd2[:], Op.bypass, Op.mult,
                        accum_out=acc[:, t // 2:t // 2 + 1])

        for p in range(NPAIRS):
            r = p // NPAIR_R
            td, ts = r >> 1, r & 1
            xc = xcb[p % XC_DEPTH]
            c0 = p * 256
            q = p % 2
            b = xc[:]
            if td == ts:
                out3 = AP(b.tensor, b.offset, [b.ap[0], [DF, 2 * TPP], [1, DF]])
                nc.gpsimd.dma_gather(
                    out_ap=out3, in_ap=tables[td][:],
                    idxs_ap=gidx_sb[:, ds(c0, 256)],
                    num_idxs=2 * EPP, num_idxs_reg=2 * EPP, elem_size=DF,
                    single_packet=False, queue_num=q)
                compute_fused(p, xc)
            else:
                outd = AP(b.tensor, b.offset, [b.ap[0], [DF, TPP], [1, DF]])
                nc.gpsimd.dma_gather(
                    out_ap=outd, in_ap=tables[td][:],
                    idxs_ap=gidx_sb[:, ds(c0, 128)],
                    num_idxs=EPP, num_idxs_reg=EPP, elem_size=DF,
                    single_packet=False, queue_num=q)
                outs = AP(b.tensor, b.offset + TPP * DF,
                          [b.ap[0], [DF, TPP], [1, DF]])
                nc.gpsimd.dma_gather(
                    out_ap=outs, in_ap=tables[ts][:],
                    idxs_ap=gidx_sb[:, ds(c0 + 128, 128)],
                    num_idxs=EPP, num_idxs_reg=EPP, elem_size=DF,
                    single_packet=False, queue_num=q)
                compute_split(p, xc)

        colsum = pp.tile([128, 1], f32, tag="colsum")
        ones = pp.tile([128, 1], f32, tag="ones")
        nc.vector.reduce_sum(out=colsum[:], in_=acc[:],
                             axis=mybir.AxisListType.X)
        nc.gpsimd.memset(ones[:], 1.0)
        pt = psp.tile([1, 1], f32, tag="pt")
        nc.tensor.matmul(pt[:], lhsT=colsum[:], rhs=ones[:],
                         start=True, stop=True)
        lsb = pp.tile([1, 1], f32, tag="lsb")
        # *2: each undirected pair contributes both directed edges equally
        nc.vector.tensor_scalar(lsb[:], pt[:], 2.0, None, Op.mult)
        nc.sync.dma_start(loss_d[:], lsb[:])

    nc.compile()
    return nc


_CACHED = {}


def _get_program():
    if "nc" not in _CACHED:
        _inject_axon_hooks()
        _CACHED["nc"] = _build_program()
    return _CACHED["nc"]


def _wrap16(v):
    """dma_gather wrapped index layout: [16, L/16] (i = s*16 + p), x8 down
    the partitions."""
    return np.tile(v.reshape(-1, 16).T, (8, 1))


def _prep_core_inputs(x_flat, maps3d, src, dst, core):
    """Build the per-core input dict (layout transforms only).
    Returns None if a bucket overflows its static region (caller falls back)."""
    e0 = core * EPC
    e1 = e0 + EPC
    d = dst[e0:e1]
    s = src[e0:e1]
    hi = maps3d[H + e0:H + e1]
    lo = maps3d[e0:e1]

    b = (d >= XS).astype(np.int32) * 2 + (s >= XS).astype(np.int32)
    counts = np.bincount(b, minlength=NREG)
    if counts.max() > REG_EDGES:
        return None
    order = np.argsort(b, kind="stable")

    # padded slot arrays; pad slots index row 0 of the region's tables and
    # carry zero maps (zero contribution; keeps every gather index valid)
    regarr = np.repeat(np.arange(NREG, dtype=np.int32), REG_EDGES)
    dp = (regarr >> 1) * XS
    sp = (regarr & 1) * XS
    hp = np.zeros((NSLOT, D, D), np.float32)
    lp = np.zeros((NSLOT, D, D), np.float32)
    pos = 0
    for r in range(NREG):
        idx = order[pos:pos + counts[r]]
        base = r * REG_EDGES
        dp[base:base + counts[r]] = d[idx]
        sp[base:base + counts[r]] = s[idx]
        hp[base:base + counts[r]] = hi[idx]
        lp[base:base + counts[r]] = lo[idx]
        pos += counts[r]

    dl = (dp - (regarr >> 1) * XS).astype(np.int16)
    sl = (sp - (regarr & 1) * XS).astype(np.int16)

    # mcat rows: [slot, i, jj]: jj<4 -> maps_hi, jj>=4 -> maps_lo (negated
    # on device)
    inter = np.empty((NSLOT, D, 8), np.float32)
    inter[:, :, :4] = hp
    inter[:, :, 4:] = lp
    mcat = inter.reshape(NT, 128, 32).transpose(1, 0, 2).reshape(128, -1)

    gidx = np.empty((128, NPAIRS * 256), np.int16)
    for p in range(NPAIRS):
        r = p // NPAIR_R
        dlp = dl[p * EPP:(p + 1) * EPP]
        slp = sl[p * EPP:(p + 1) * EPP]
        c0 = p * 256
        if (r >> 1) == (r & 1):
            streamd = np.stack(
                [dlp.reshape(TPP, 128), slp.reshape(TPP, 128)], axis=1)
            gidx[:, c0:c0 + 256] = _wrap16(streamd.reshape(-1))
        else:
            gidx[:, c0:c0 + 128] = _wrap16(dlp)
            gidx[:, c0 + 128:c0 + 256] = _wrap16(slp)

    return {
        "mcat": np.ascontiguousarray(mcat),
        "gidx": np.ascontiguousarray(gidx),
    }


def _symmetric_structure(rev_idx):
    r = np.asarray(rev_idx)
    if r.shape != (E,):
        return False
    h = np.arange(H, dtype=r.dtype)
    return bool(np.array_equal(r[:H], h + H) and np.array_equal(r[H:], h))


def _fallback_numpy(x, restriction_maps, edge_index, rev_idx):
    x = np.asarray(x, np.float32)
    maps = np.asarray(restriction_maps, np.float32)
    ei = np.asarray(edge_index)
    rv = np.asarray(rev_idx)
    total = np.float64(0.0)
    chunk = 131072
    ne = ei.shape[1]
    for st in range(0, ne, chunk):
        e = min(st + chunk, ne)
        src = ei[0, st:e]
        tgt = ei[1, st:e]
        fvu = maps[rv[st:e]]
        fuv = maps[st:e]
        t1 = np.einsum("eij,ejf->eif", fvu, x[tgt])
        t2 = np.einsum("eij,ejf->eif", fuv, x[src])
        dm = t1 - t2
        total += np.sum((dm * dm).astype(np.float64))
    return np.float32(total)


def kernel(x, restriction_maps, edge_index, rev_idx):
    x = np.asarray(x)
    restriction_maps = np.asarray(restriction_maps)
    edge_index = np.asarray(edge_index)
    rev_idx = np.asarray(rev_idx)

    if (x.shape != (N, D, F) or restriction_maps.shape != (E, D, D)
            or edge_index.shape != (2, E) or not _symmetric_structure(rev_idx)):
        return _fallback_numpy(x, restriction_maps, edge_index, rev_idx)

    x_flat = x.reshape(N, DF).astype(np.float32)
    maps3d = restriction_maps.astype(np.float32)
    src = edge_index[0].astype(np.int32)
    dst = edge_index[1].astype(np.int32)

    in_maps = []
    for c in range(NCORES):
        m = _prep_core_inputs(x_flat, maps3d, src, dst, c)
        if m is None:
            return _fallback_numpy(x, restriction_maps, edge_index, rev_idx)
        m["xlo"] = np.ascontiguousarray(x_flat[:XS])
        m["xhi"] = np.ascontiguousarray(x_flat[XS:])
        in_maps.append(m)

    from concourse.bass_utils import run_bass_kernel_spmd

    nc = _get_program()
    res = run_bass_kernel_spmd(nc, in_maps, core_ids=list(range(NCORES)))
    total = np.float32(0.0)
    for c in range(NCORES):
        total += res.results[c]["loss"][0, 0]
    return np.float32(total)
